# revision 9
# baseline (speedup 1.0000x reference)
"""Trainium2 Bass kernel for nn_Decoder_28922309771884 (sparse_attention decoder layer).

Strategy (8 NeuronCores):
  - Head-parallel attention: 32 heads / 8 cores = 4 heads per core.
    QKV projections column-sharded, wo row-sharded -> per-core partial o.
  - MLP tensor-parallel on the 11008 ffn dim (1376/core, zero-padded to 1408).
  - Two SPMD launches; the cross-core reductions (sum of o partials, sum of
    mlp partials) and the cheap row-stat math (rmsnorm scales, residual adds,
    transposes) run on host between launches.  Device collectives measured
    ~3.6 ms per 16 MiB AllReduce here - far slower than host reduction.
  - All matmuls run with bf16 inputs (fp32 PSUM accumulation).  fp32 data
    feeds every non-matmul stage.
"""

import sys

sys.path.insert(0, "/opt/trn_rl_repo")

import numpy as np
import ml_dtypes

import concourse.bass as bass
import concourse.mybir as mybir
import concourse.tile as tile
from concourse import bacc
from concourse.masks import make_identity

f32 = mybir.dt.float32
f32r = mybir.dt.float32r
bf16 = mybir.dt.bfloat16
BF = ml_dtypes.bfloat16

B, S, H, NH, HD, FF = 1, 1024, 4096, 32, 128, 11008
GAMMA = 64.0
EPS = 1e-5
NC = 8                 # cores
HPC = NH // NC         # heads per core = 4
DPC = HPC * HD         # head dims per core = 512
FFP = 1408             # padded ffn dims per core (1376 -> 11*128)
ISQ = float(1.0 / np.sqrt(HD))
SCH = S // 128         # 8 sequence chunks
KCH = H // 128         # 32 hidden chunks
NEG = -3.0e38


# ---------------------------------------------------------------- L1 program
def _build_l1():
    nc = bacc.Bacc("TRN2", target_bir_lowering=False, debug=False, num_devices=NC)
    hid = nc.dram_tensor("hid", [S, H], f32, kind="ExternalInput").ap()
    invr = nc.dram_tensor("invr", [S], f32, kind="ExternalInput").ap()
    invc = nc.dram_tensor("invc", [128, SCH], f32, kind="ExternalInput").ap()
    wq = nc.dram_tensor("wq", [H, DPC], f32, kind="ExternalInput").ap()
    wk = nc.dram_tensor("wk", [H, DPC], f32, kind="ExternalInput").ap()
    wv = nc.dram_tensor("wv", [H, DPC], f32, kind="ExternalInput").ap()
    wo = nc.dram_tensor("wo", [DPC, H], bf16, kind="ExternalInput").ap()
    rot1 = nc.dram_tensor("rot1", [HPC, HD, HD], f32, kind="ExternalInput").ap()
    rot2 = nc.dram_tensor("rot2", [HPC, HD, HD], f32, kind="ExternalInput").ap()
    csB = nc.dram_tensor("csB", [HD, S], f32, kind="ExternalInput").ap()
    snB = nc.dram_tensor("snB", [HD, S], f32, kind="ExternalInput").ap()
    tri = nc.dram_tensor("tri", [128, 128], f32, kind="ExternalInput").ap()

    draft = nc.dram_tensor("draft", [HPC, S, S], f32, kind="ExternalOutput").ap()
    trueo = nc.dram_tensor("trueo", [HPC, S, S], f32, kind="ExternalOutput").ap()
    opart = nc.dram_tensor("opart", [S, H], f32, kind="ExternalOutput").ap()

    with tile.TileContext(nc) as tc:
        import contextlib
        with contextlib.ExitStack() as ctx:
            cp = ctx.enter_context(tc.tile_pool(name="cp", bufs=1))
            pers = ctx.enter_context(tc.tile_pool(name="pers", bufs=1))
            hseg = ctx.enter_context(tc.tile_pool(name="hseg", bufs=1))
            hcp = ctx.enter_context(tc.tile_pool(name="hcp", bufs=4))
            wp = ctx.enter_context(tc.tile_pool(name="wp", bufs=4))
            wop = ctx.enter_context(tc.tile_pool(name="wop", bufs=5))
            sc1 = ctx.enter_context(tc.tile_pool(name="sc1", bufs=4))
            sc2 = ctx.enter_context(tc.tile_pool(name="sc2", bufs=2))
            bigf = ctx.enter_context(tc.tile_pool(name="bigf", bufs=5))
            pp2 = ctx.enter_context(tc.tile_pool(name="pp2", bufs=3, space="PSUM"))
            ppt = ctx.enter_context(tc.tile_pool(name="ppt", bufs=3, space="PSUM"))
            ppa = ctx.enter_context(tc.tile_pool(name="ppa", bufs=1, space="PSUM"))

            # ---------------- constants
            idb = cp.tile([128, 128], bf16)
            make_identity(nc, idb)
            idf = cp.tile([128, 128], f32)
            make_identity(nc, idf)
            idr = cp.tile([128, 128], f32r)
            nc.gpsimd.tensor_copy(out=idr, in_=idf)
            csB_sb = cp.tile([128, S], f32)
            nc.sync.dma_start(csB_sb, csB)
            snB_sb = cp.tile([128, S], f32)
            nc.sync.dma_start(snB_sb, snB)
            tri_sb = cp.tile([128, 128], f32)
            nc.sync.dma_start(tri_sb, tri)
            invc_sb = cp.tile([128, SCH], f32)
            nc.sync.dma_start(invc_sb, invc)
            zt = cp.tile([128, S - 128], f32)
            nc.vector.memset(zt, 0.0)

            # persistent activation tiles (per-partition bytes in comments)
            qrT = pers.tile([128, HPC, S], f32r)         # 16K
            krT = pers.tile([128, HPC, S], f32r)         # 16K
            v_sb = pers.tile([128, SCH, DPC], bf16)      # 8K
            attnT = pers.tile([128, HPC, S], bf16)       # 8K

            # ---------------- phase 1+2: QKV in f32r, S processed in halves
            for g in range(2):
                hidT = hseg.tile([128, KCH, 512], f32r, tag="hidT")
                for s4 in range(4):
                    s = g * 4 + s4
                    for k in range(KCH):
                        hc = hcp.tile([128, 128], f32, tag="hc")
                        nc.sync.dma_start(
                            hc, hid[s * 128:(s + 1) * 128, k * 128:(k + 1) * 128]
                        )
                        hr = hcp.tile([128, 128], f32r, tag="hr")
                        nc.gpsimd.tensor_copy(out=hr, in_=hc)
                        ptr = ppt.tile([128, 128], f32r, tag="pt")
                        nc.tensor.transpose(ptr, hr, idr)
                        nc.vector.tensor_copy(
                            out=hidT[:, k, s4 * 128:(s4 + 1) * 128], in_=ptr
                        )
                gs = slice(g * 512, (g + 1) * 512)
                for widx, wap in ((0, wq), (1, wk), (2, wv)):
                    for m in range(HPC):
                        pb = pp2.tile([128, 512], f32, tag="pb")
                        for k in range(KCH):
                            ws = wp.tile([128, 128], f32, tag="ws")
                            nc.sync.dma_start(
                                ws, wap[k * 128:(k + 1) * 128, m * 128:(m + 1) * 128]
                            )
                            wr = wp.tile([128, 128], f32r, tag="wr")
                            nc.gpsimd.tensor_copy(out=wr, in_=ws)
                            nc.tensor.matmul(pb, wr, hidT[:, k, :],
                                             start=(k == 0), stop=(k == KCH - 1))
                        if widx < 2:
                            # rope: qr = q*csB + swap_halves(q)*snB
                            qraw = sc1.tile([128, 512], f32, tag="scr1")
                            nc.vector.tensor_copy(out=qraw, in_=pb)
                            tcos = sc1.tile([128, 512], f32, tag="scr1")
                            nc.vector.tensor_mul(out=tcos, in0=pb, in1=csB_sb[:, gs])
                            qsw = sc1.tile([128, 512], f32, tag="scr1")
                            nc.sync.dma_start(qsw[0:64, :], qraw[64:128, :])
                            nc.sync.dma_start(qsw[64:128, :], qraw[0:64, :])
                            tsin = sc1.tile([128, 512], f32, tag="scr1")
                            nc.vector.tensor_mul(out=tsin, in0=qsw, in1=snB_sb[:, gs])
                            dst = qrT if widx == 0 else krT
                            nc.vector.tensor_add(out=dst[:, m, gs], in0=tcos, in1=tsin)
                        else:
                            # v: cast bf16, transpose to [s, hd], normalize rows
                            vb = sc2.tile([128, 512], bf16, tag="vb")
                            nc.vector.tensor_copy(out=vb, in_=pb)
                            for s4 in range(4):
                                s = g * 4 + s4
                                ptv = ppt.tile([128, 128], bf16, tag="pt")
                                nc.tensor.transpose(
                                    ptv, vb[:, s4 * 128:(s4 + 1) * 128], idb
                                )
                                nc.vector.tensor_scalar_mul(
                                    out=v_sb[:, s, m * 128:(m + 1) * 128],
                                    in0=ptv,
                                    scalar1=invc_sb[:, s:s + 1],
                                )

            # ---------------- phase 3: hash scores (draft)
            for h in range(HPC):
                rt1s = wp.tile([128, 128], f32, tag="rts")
                nc.sync.dma_start(rt1s, rot1[h])
                rt1 = wp.tile([128, 128], f32r, tag="rt")
                nc.gpsimd.tensor_copy(out=rt1, in_=rt1s)
                rt2s = wp.tile([128, 128], f32, tag="rts")
                nc.sync.dma_start(rt2s, rot2[h])
                rt2 = wp.tile([128, 128], f32r, tag="rt")
                nc.gpsimd.tensor_copy(out=rt2, in_=rt2s)
                hashT = {}
                for side, srcT in (("q", qrT), ("k", krT)):
                    hh = sc2.tile([128, S], f32r, tag=f"hash{side}")
                    for g in range(2):
                        gs = slice(g * 512, (g + 1) * 512)
                        hp1 = pp2.tile([128, 512], f32, tag="pb")
                        nc.tensor.matmul(hp1, rt1, srcT[:, h, gs],
                                         start=True, stop=True)
                        s1 = sc2.tile([128, 512], f32r, tag="s1")
                        nc.scalar.activation(out=s1, in_=hp1,
                                             func=mybir.ActivationFunctionType.Silu)
                        hp2 = pp2.tile([128, 512], f32, tag="pb")
                        nc.tensor.matmul(hp2, rt2, s1, start=True, stop=True)
                        ab = sc1.tile([128, 512], f32, tag="scr1")
                        nc.scalar.activation(out=ab, in_=hp2,
                                             func=mybir.ActivationFunctionType.Abs,
                                             scale=GAMMA)
                        nc.scalar.add(out=ab, in_=ab, add=1.0)
                        rcp = sc1.tile([128, 512], f32, tag="scr1")
                        nc.vector.reciprocal(out=rcp, in_=ab)
                        nc.scalar.mul(out=rcp, in_=rcp, mul=GAMMA)
                        nc.vector.tensor_mul(out=hh[:, gs], in0=hp2, in1=rcp)
                    hashT[side] = hh
                for qc in range(SCH):
                    for g in range(2):
                        dp = pp2.tile([128, 512], f32, tag="pb")
                        nc.tensor.matmul(
                            dp, hashT["q"][:, qc * 128:(qc + 1) * 128],
                            hashT["k"][:, g * 512:(g + 1) * 512],
                            start=True, stop=True,
                        )
                        dcp = bigf.tile([128, 512], f32, tag="bigf")
                        nc.vector.tensor_copy(out=dcp, in_=dp)
                        nc.sync.dma_start(
                            draft[h, qc * 128:(qc + 1) * 128,
                                  g * 512:(g + 1) * 512], dcp
                        )

            # ---------------- phase 4: true causal attention
            for h in range(HPC):
                for qc in range(SCH):
                    ks = (qc + 1) * 128
                    lsb = bigf.tile([128, S], f32, tag="bigf")
                    for n0 in range(0, ks, 512):
                        nn = min(512, ks - n0)
                        lp = pp2.tile([128, 512], f32, tag="pb")
                        nc.tensor.matmul(
                            lp[:, 0:nn],
                            qrT[:, h, qc * 128:(qc + 1) * 128],
                            krT[:, h, n0:n0 + nn], start=True, stop=True,
                        )
                        nc.vector.tensor_copy(out=lsb[:, n0:n0 + nn], in_=lp[:, 0:nn])
                    nc.vector.tensor_add(
                        out=lsb[:, qc * 128:ks], in0=lsb[:, qc * 128:ks], in1=tri_sb
                    )
                    negm = sc2.tile([128, 1], f32, tag="negm")
                    nc.vector.reduce_max(out=negm, in_=lsb[:, 0:ks],
                                         axis=mybir.AxisListType.X, negate=True)
                    negms = sc2.tile([128, 1], f32, tag="negms")
                    nc.scalar.mul(out=negms, in_=negm, mul=ISQ)
                    pu = bigf.tile([128, S], f32, tag="bigf")
                    rsum = sc2.tile([128, 1], f32, tag="rsum")
                    nc.scalar.activation(out=pu[:, 0:ks], in_=lsb[:, 0:ks],
                                         func=mybir.ActivationFunctionType.Exp,
                                         bias=negms, scale=ISQ, accum_out=rsum)
                    rinv = sc2.tile([128, 1], f32, tag="rinv")
                    nc.vector.reciprocal(out=rinv, in_=rsum)
                    pf = bigf.tile([128, S], f32, tag="bigf")
                    nc.vector.tensor_scalar_mul(out=pf[:, 0:ks], in0=pu[:, 0:ks],
                                                scalar1=rinv)
                    nc.sync.dma_start(trueo[h, qc * 128:(qc + 1) * 128, 0:ks],
                                      pf[:, 0:ks])
                    if ks < S:
                        nc.sync.dma_start(trueo[h, qc * 128:(qc + 1) * 128, ks:S],
                                          zt[:, 0:S - ks])
                    pbf = sc2.tile([128, S], bf16, tag="pbf")
                    nc.vector.tensor_scalar_mul(out=pbf[:, 0:ks], in0=pu[:, 0:ks],
                                                scalar1=rinv)
                    ap_ps = ppa.tile([128, 128], f32, tag="ap_ps")
                    for kc in range(qc + 1):
                        ptp = ppt.tile([128, 128], bf16, tag="pt")
                        nc.tensor.transpose(ptp, pbf[:, kc * 128:(kc + 1) * 128], idb)
                        pts = sc2.tile([128, 128], bf16, tag="pts")
                        nc.vector.tensor_copy(out=pts, in_=ptp)
                        nc.tensor.matmul(
                            ap_ps, v_sb[:, kc, h * 128:(h + 1) * 128], pts,
                            start=(kc == 0), stop=(kc == qc),
                        )
                    nc.vector.tensor_copy(
                        out=attnT[:, h, qc * 128:(qc + 1) * 128], in_=ap_ps
                    )

            # ---------------- phase 5: output projection partial
            for n in range(8):
                wots = []
                for hh in range(HPC):
                    wot = wop.tile([128, 512], bf16, tag="wot")
                    nc.sync.dma_start(
                        wot, wo[hh * 128:(hh + 1) * 128, n * 512:(n + 1) * 512]
                    )
                    wots.append(wot)
                for sc in range(SCH):
                    po = pp2.tile([128, 512], f32, tag="pb")
                    for hh in range(HPC):
                        nc.tensor.matmul(
                            po, attnT[:, hh, sc * 128:(sc + 1) * 128], wots[hh],
                            start=(hh == 0), stop=(hh == HPC - 1),
                        )
                    ob = sc2.tile([128, 512], f32, tag="ob")
                    nc.scalar.copy(out=ob, in_=po)
                    nc.sync.dma_start(
                        opart[sc * 128:(sc + 1) * 128, n * 512:(n + 1) * 512], ob
                    )

    nc.compile()
    return nc


# ---------------------------------------------------------------- L2 program
def _build_l2():
    nc = bacc.Bacc("TRN2", target_bir_lowering=False, debug=False, num_devices=NC)
    hnT = nc.dram_tensor("hnT", [H, S], bf16, kind="ExternalInput").ap()
    wg = nc.dram_tensor("wg", [H, FFP], bf16, kind="ExternalInput").ap()
    wu = nc.dram_tensor("wu", [H, FFP], bf16, kind="ExternalInput").ap()
    wd = nc.dram_tensor("wd", [FFP, H], bf16, kind="ExternalInput").ap()
    mlp = nc.dram_tensor("mlp", [S, H], f32, kind="ExternalOutput").ap()
    FCH = FFP // 128  # 11

    with tile.TileContext(nc) as tc:
        import contextlib
        with contextlib.ExitStack() as ctx:
            pers = ctx.enter_context(tc.tile_pool(name="pers", bufs=1))
            wp = ctx.enter_context(tc.tile_pool(name="wp", bufs=4))
            wdp = ctx.enter_context(tc.tile_pool(name="wdp", bufs=12))
            sc2 = ctx.enter_context(tc.tile_pool(name="sc2", bufs=2))
            bigf = ctx.enter_context(tc.tile_pool(name="bigf", bufs=5))
            pp2 = ctx.enter_context(tc.tile_pool(name="pp2", bufs=2, space="PSUM"))
            ppo = ctx.enter_context(tc.tile_pool(name="ppo", bufs=2, space="PSUM"))

            hnT_sb = pers.tile([128, KCH, S], bf16)   # 64K
            gated = pers.tile([128, FCH, S], bf16)    # 22K
            for k in range(KCH):
                nc.sync.dma_start(hnT_sb[:, k, :], hnT[k * 128:(k + 1) * 128, :])

            for f in range(FCH):
                pg = pp2.tile([128, S], f32, tag="pg")
                for k in range(KCH):
                    wgt = wp.tile([128, 128], bf16, tag="wgt")
                    nc.sync.dma_start(
                        wgt, wg[k * 128:(k + 1) * 128, f * 128:(f + 1) * 128]
                    )
                    for n0 in (0, 512):
                        nc.tensor.matmul(pg[:, n0:n0 + 512], wgt,
                                         hnT_sb[:, k, n0:n0 + 512],
                                         start=(k == 0), stop=(k == KCH - 1))
                sg = sc2.tile([128, S], bf16, tag="sg")
                nc.scalar.activation(out=sg, in_=pg,
                                     func=mybir.ActivationFunctionType.Silu)
                pup = pp2.tile([128, S], f32, tag="pg")
                for k in range(KCH):
                    wut = wp.tile([128, 128], bf16, tag="wgt")
                    nc.sync.dma_start(
                        wut, wu[k * 128:(k + 1) * 128, f * 128:(f + 1) * 128]
                    )
                    for n0 in (0, 512):
                        nc.tensor.matmul(pup[:, n0:n0 + 512], wut,
                                         hnT_sb[:, k, n0:n0 + 512],
                                         start=(k == 0), stop=(k == KCH - 1))
                nc.vector.tensor_mul(out=gated[:, f, :], in0=pup, in1=sg)

            for n in range(8):
                wdts = []
                for f in range(FCH):
                    wdt = wdp.tile([128, 512], bf16, tag="wdt")
                    nc.sync.dma_start(
                        wdt, wd[f * 128:(f + 1) * 128, n * 512:(n + 1) * 512]
                    )
                    wdts.append(wdt)
                for sc in range(SCH):
                    pd = ppo.tile([128, 512], f32, tag="pd")
                    for f in range(FCH):
                        nc.tensor.matmul(
                            pd, gated[:, f, sc * 128:(sc + 1) * 128], wdts[f],
                            start=(f == 0), stop=(f == FCH - 1),
                        )
                    ob = sc2.tile([128, 512], f32, tag="ob")
                    nc.scalar.copy(out=ob, in_=pd)
                    nc.sync.dma_start(
                        mlp[sc * 128:(sc + 1) * 128, n * 512:(n + 1) * 512], ob
                    )

    nc.compile()
    return nc


# ------------------------------------------------------- cached SPMD runner
class _SpmdRunner:
    """Compile a Bass program into a cached jax.jit callable over 8 cores."""

    def __init__(self, nc):
        import jax
        from jax.experimental.shard_map import shard_map
        from jax.sharding import Mesh, PartitionSpec
        from concourse import bass2jax

        bass2jax.install_neuronx_cc_hook()
        part_name = nc.partition_id_tensor.name if nc.partition_id_tensor else None
        self.in_names, self.out_names, out_avals, self.zero_outs = [], [], [], []
        for alloc in nc.m.functions[0].allocations:
            if not isinstance(alloc, mybir.MemoryLocationSet):
                continue
            name = alloc.memorylocations[0].name
            if alloc.kind == "ExternalInput":
                if name != part_name:
                    self.in_names.append(name)
            elif alloc.kind == "ExternalOutput":
                shape = tuple(alloc.tensor_shape)
                dtype = mybir.dt.np(alloc.dtype)
                out_avals.append(jax.core.ShapedArray(shape, dtype))
                self.out_names.append(name)
                self.zero_outs.append(np.zeros(shape, dtype))
        self.out_avals = out_avals
        n_params = len(self.in_names)
        all_names = self.in_names + self.out_names
        if part_name is not None:
            all_names = all_names + [part_name]

        def _body(*args):
            operands = list(args)
            if part_name is not None:
                operands.append(bass2jax.partition_id_tensor())
            outs = bass2jax._bass_exec_p.bind(
                *operands,
                out_avals=tuple(out_avals),
                in_names=tuple(all_names),
                out_names=tuple(self.out_names),
                lowering_input_output_aliases=(),
                sim_require_finite=True,
                sim_require_nnan=True,
                nc=nc,
            )
            return tuple(outs)

        devices = jax.devices()[:NC]
        mesh = Mesh(np.asarray(devices), ("core",))
        n_all = n_params + len(self.out_names)
        self.fn = jax.jit(
            shard_map(
                _body, mesh=mesh,
                in_specs=(PartitionSpec("core"),) * n_all,
                out_specs=(PartitionSpec("core"),) * len(self.out_names),
                check_rep=False,
            ),
            donate_argnums=tuple(range(n_params, n_all)),
            keep_unused=True,
        )

    def __call__(self, in_maps):
        concat_in = [
            np.concatenate([np.asarray(m[name]) for m in in_maps], axis=0)
            for name in self.in_names
        ]
        concat_zero = [
            np.zeros((NC * z.shape[0], *z.shape[1:]), z.dtype) for z in self.zero_outs
        ]
        outs = self.fn(*concat_in, *concat_zero)
        return [
            {
                name: np.asarray(outs[i]).reshape(NC, *self.out_avals[i].shape)[c]
                for i, name in enumerate(self.out_names)
            }
            for c in range(NC)
        ]


_CACHE = {}


def _programs():
    if "r1" not in _CACHE:
        _CACHE["r1"] = _SpmdRunner(_build_l1())
        _CACHE["r2"] = _SpmdRunner(_build_l2())
    return _CACHE["r1"], _CACHE["r2"]


# ----------------------------------------------------------- host-side prep
def _rope_cos_sin_T():
    inv_freq = 1.0 / (10000.0 ** (np.arange(0, HD, 2, dtype=np.float64) / HD))
    freqs = np.outer(np.arange(S, dtype=np.float64), inv_freq)
    emb = np.concatenate([freqs, freqs], axis=-1)
    return (np.cos(emb).astype(np.float32).T.copy(),
            np.sin(emb).astype(np.float32).T.copy())


def _l1_inputs(hidden, wq, wk, wv, wo, rot1, rot2, ln1_w):
    hid = np.asarray(hidden, np.float32).reshape(S, H)
    invr = (1.0 / np.sqrt((hid.astype(np.float64) ** 2).mean(-1) + EPS)).astype(
        np.float32
    )
    invc = invr.reshape(SCH, 128).T.copy()          # [i, s]
    cosT, sinT = _rope_cos_sin_T()
    csB = (cosT * invr[None, :]).astype(np.float32)
    snB = (sinT * invr[None, :]).astype(np.float32)
    snB[0:64, :] *= -1.0
    tri = np.triu(np.full((128, 128), NEG, np.float32), k=1)
    wq = (np.asarray(wq, np.float32) * np.asarray(ln1_w, np.float32)[:, None])
    wk = (np.asarray(wk, np.float32) * np.asarray(ln1_w, np.float32)[:, None])
    wv = (np.asarray(wv, np.float32) * np.asarray(ln1_w, np.float32)[:, None])
    wo = np.asarray(wo, np.float32)
    r1 = np.asarray(rot1, np.float32).reshape(NH, HD, HD)
    r2 = np.asarray(rot2, np.float32).reshape(NH, HD, HD)
    maps = []
    for c in range(NC):
        cs = slice(c * DPC, (c + 1) * DPC)
        hs = slice(c * HPC, (c + 1) * HPC)
        maps.append({
            "hid": hid,
            "invr": invr,
            "invc": invc,
            "wq": np.ascontiguousarray(wq[:, cs]),
            "wk": np.ascontiguousarray(wk[:, cs]),
            "wv": np.ascontiguousarray(wv[:, cs]),
            "wo": wo[cs, :].astype(BF),
            "rot1": r1[hs],
            "rot2": r2[hs],
            "csB": csB,
            "snB": snB,
            "tri": tri,
        })
    return maps, hid


def _l2_inputs(h, ln2_w, w_gate, w_up, w_down):
    hn = h * (1.0 / np.sqrt((h.astype(np.float64) ** 2).mean(-1, keepdims=True)
                            + EPS)).astype(np.float32)
    hn = hn * np.asarray(ln2_w, np.float32)[None, :]
    hnT = np.ascontiguousarray(hn.T).astype(BF)
    wg = np.asarray(w_gate, np.float32)
    wu = np.asarray(w_up, np.float32)
    wd = np.asarray(w_down, np.float32)
    maps = []
    for c in range(NC):
        f0 = c * (FF // NC)
        f1 = (c + 1) * (FF // NC)
        wgp = np.zeros((H, FFP), BF)
        wgp[:, : FF // NC] = wg[:, f0:f1].astype(BF)
        wup = np.zeros((H, FFP), BF)
        wup[:, : FF // NC] = wu[:, f0:f1].astype(BF)
        wdp = np.zeros((FFP, H), BF)
        wdp[: FF // NC, :] = wd[f0:f1, :].astype(BF)
        maps.append({"hnT": hnT, "wg": wgp, "wu": wup, "wd": wdp})
    return maps


# ------------------------------------------------------------------- kernel
def kernel(hidden_states, wq, wk, wv, wo, rot_mat1, rot_mat2, ln1_w, ln2_w,
           w_gate, w_up, w_down):
    r1, r2 = _programs()

    maps1, hid = _l1_inputs(hidden_states, wq, wk, wv, wo, rot_mat1, rot_mat2,
                            ln1_w)
    res1 = r1(maps1)

    o_sum = np.sum(np.stack([r["opart"] for r in res1]), axis=0, dtype=np.float32)
    h = hid + o_sum
    maps2 = _l2_inputs(h, ln2_w, w_gate, w_up, w_down)
    res2 = r2(maps2)

    mlp_sum = np.sum(np.stack([r["mlp"] for r in res2]), axis=0, dtype=np.float32)
    out1 = (h + mlp_sum).reshape(B, S, H)
    draft = np.concatenate([r["draft"] for r in res1], axis=0).reshape(B, NH, S, S)
    true = np.concatenate([r["trueo"] for r in res1], axis=0).reshape(B, NH, S, S)
    return out1, draft, true


# revision 10
# speedup vs baseline: 538.1552x; 538.1552x over previous
"""Trainium2 Bass kernel for nn_Decoder_28922309771884 (sparse_attention decoder layer).

Strategy (8 NeuronCores):
  - Head-parallel attention: 32 heads / 8 cores = 4 heads per core.
    QKV projections column-sharded, wo row-sharded -> per-core partial o.
  - MLP tensor-parallel on the 11008 ffn dim (1376/core, zero-padded to 1408).
  - Two SPMD launches; the cross-core reductions (sum of o partials, sum of
    mlp partials) and the cheap row-stat math (rmsnorm scales, residual adds,
    transposes) run on host between launches.  Device collectives measured
    ~3.6 ms per 16 MiB AllReduce here - far slower than host reduction.
  - All matmuls run with bf16 inputs (fp32 PSUM accumulation).  fp32 data
    feeds every non-matmul stage.
"""

import sys

sys.path.insert(0, "/opt/trn_rl_repo")

import numpy as np
import ml_dtypes

import concourse.bass as bass
import concourse.mybir as mybir
import concourse.tile as tile
from concourse import bacc
from concourse.masks import make_identity

f32 = mybir.dt.float32
f32r = mybir.dt.float32r
bf16 = mybir.dt.bfloat16
BF = ml_dtypes.bfloat16

B, S, H, NH, HD, FF = 1, 1024, 4096, 32, 128, 11008
GAMMA = 64.0
EPS = 1e-5
NC = 8                 # cores
HPC = NH // NC         # heads per core = 4
DPC = HPC * HD         # head dims per core = 512
FFP = 1408             # padded ffn dims per core (1376 -> 11*128)
ISQ = float(1.0 / np.sqrt(HD))
SCH = S // 128         # 8 sequence chunks
KCH = H // 128         # 32 hidden chunks
NEG = -3.0e38


# ---------------------------------------------------------------- L1 program
def _build_l1(reps: int = 1):
    nc = bacc.Bacc("TRN2", target_bir_lowering=False, debug=False, num_devices=NC)
    hid = nc.dram_tensor("hid", [S, H], f32, kind="ExternalInput").ap()
    invr = nc.dram_tensor("invr", [S], f32, kind="ExternalInput").ap()
    invc = nc.dram_tensor("invc", [128, SCH], f32, kind="ExternalInput").ap()
    wq = nc.dram_tensor("wq", [H, DPC], f32, kind="ExternalInput").ap()
    wk = nc.dram_tensor("wk", [H, DPC], f32, kind="ExternalInput").ap()
    wv = nc.dram_tensor("wv", [H, DPC], f32, kind="ExternalInput").ap()
    wo = nc.dram_tensor("wo", [DPC, H], bf16, kind="ExternalInput").ap()
    rot1 = nc.dram_tensor("rot1", [HPC, HD, HD], f32, kind="ExternalInput").ap()
    rot2 = nc.dram_tensor("rot2", [HPC, HD, HD], f32, kind="ExternalInput").ap()
    csB = nc.dram_tensor("csB", [HD, S], f32, kind="ExternalInput").ap()
    snB = nc.dram_tensor("snB", [HD, S], f32, kind="ExternalInput").ap()
    tri = nc.dram_tensor("tri", [128, 128], f32, kind="ExternalInput").ap()

    draft = nc.dram_tensor("draft", [HPC, S, S], f32, kind="ExternalOutput").ap()
    trueo = nc.dram_tensor("trueo", [HPC, S, S], f32, kind="ExternalOutput").ap()
    opart = nc.dram_tensor("opart", [S, H], f32, kind="ExternalOutput").ap()

    with tile.TileContext(nc) as tc:
        import contextlib
        with contextlib.ExitStack() as ctx:
            cp = ctx.enter_context(tc.tile_pool(name="cp", bufs=1))
            pers = ctx.enter_context(tc.tile_pool(name="pers", bufs=1))
            hseg = ctx.enter_context(tc.tile_pool(name="hseg", bufs=1))
            hcp = ctx.enter_context(tc.tile_pool(name="hcp", bufs=4))
            wp = ctx.enter_context(tc.tile_pool(name="wp", bufs=4))
            wop = ctx.enter_context(tc.tile_pool(name="wop", bufs=5))
            sc1 = ctx.enter_context(tc.tile_pool(name="sc1", bufs=4))
            sc2 = ctx.enter_context(tc.tile_pool(name="sc2", bufs=2))
            bigf = ctx.enter_context(tc.tile_pool(name="bigf", bufs=5))
            pp2 = ctx.enter_context(tc.tile_pool(name="pp2", bufs=3, space="PSUM"))
            ppt = ctx.enter_context(tc.tile_pool(name="ppt", bufs=3, space="PSUM"))
            ppa = ctx.enter_context(tc.tile_pool(name="ppa", bufs=1, space="PSUM"))

            # ---------------- constants
            idb = cp.tile([128, 128], bf16)
            make_identity(nc, idb)
            idf = cp.tile([128, 128], f32)
            make_identity(nc, idf)
            idr = cp.tile([128, 128], f32r)
            nc.gpsimd.tensor_copy(out=idr, in_=idf)
            csB_sb = cp.tile([128, S], f32)
            nc.sync.dma_start(csB_sb, csB)
            snB_sb = cp.tile([128, S], f32)
            nc.sync.dma_start(snB_sb, snB)
            tri_sb = cp.tile([128, 128], f32)
            nc.sync.dma_start(tri_sb, tri)
            invc_sb = cp.tile([128, SCH], f32)
            nc.sync.dma_start(invc_sb, invc)
            zt = cp.tile([128, S - 128], f32)
            nc.vector.memset(zt, 0.0)

            # persistent activation tiles (per-partition bytes in comments)
            qrT = pers.tile([128, HPC, S], f32r)         # 16K
            krT = pers.tile([128, HPC, S], f32r)         # 16K
            v_sb = pers.tile([128, SCH, DPC], bf16)      # 8K
            attnT = pers.tile([128, HPC, S], bf16)       # 8K

            if reps > 1:
                ctx.enter_context(tc.For_i(0, reps, 1))

            # ---------------- phase 1+2: QKV in f32r, S processed in halves
            for g in range(2):
                hidT = hseg.tile([128, KCH, 512], f32r, tag="hidT")
                for s4 in range(4):
                    s = g * 4 + s4
                    for k in range(KCH):
                        hc = hcp.tile([128, 128], f32, tag="hc")
                        nc.sync.dma_start(
                            hc, hid[s * 128:(s + 1) * 128, k * 128:(k + 1) * 128]
                        )
                        hr = hcp.tile([128, 128], f32r, tag="hr")
                        nc.gpsimd.tensor_copy(out=hr, in_=hc)
                        ptr = ppt.tile([128, 128], f32r, tag="pt")
                        nc.tensor.transpose(ptr, hr, idr)
                        nc.vector.tensor_copy(
                            out=hidT[:, k, s4 * 128:(s4 + 1) * 128], in_=ptr
                        )
                gs = slice(g * 512, (g + 1) * 512)
                for widx, wap in ((0, wq), (1, wk), (2, wv)):
                    for m in range(HPC):
                        pb = pp2.tile([128, 512], f32, tag="pb")
                        for k in range(KCH):
                            ws = wp.tile([128, 128], f32, tag="ws")
                            nc.sync.dma_start(
                                ws, wap[k * 128:(k + 1) * 128, m * 128:(m + 1) * 128]
                            )
                            wr = wp.tile([128, 128], f32r, tag="wr")
                            nc.gpsimd.tensor_copy(out=wr, in_=ws)
                            nc.tensor.matmul(pb, wr, hidT[:, k, :],
                                             start=(k == 0), stop=(k == KCH - 1))
                        if widx < 2:
                            # rope: qr = q*csB + swap_halves(q)*snB
                            qraw = sc1.tile([128, 512], f32, tag="scr1")
                            nc.vector.tensor_copy(out=qraw, in_=pb)
                            tcos = sc1.tile([128, 512], f32, tag="scr1")
                            nc.vector.tensor_mul(out=tcos, in0=pb, in1=csB_sb[:, gs])
                            qsw = sc1.tile([128, 512], f32, tag="scr1")
                            nc.sync.dma_start(qsw[0:64, :], qraw[64:128, :])
                            nc.sync.dma_start(qsw[64:128, :], qraw[0:64, :])
                            tsin = sc1.tile([128, 512], f32, tag="scr1")
                            nc.vector.tensor_mul(out=tsin, in0=qsw, in1=snB_sb[:, gs])
                            dst = qrT if widx == 0 else krT
                            nc.vector.tensor_add(out=dst[:, m, gs], in0=tcos, in1=tsin)
                        else:
                            # v: cast bf16, transpose to [s, hd], normalize rows
                            vb = sc2.tile([128, 512], bf16, tag="vb")
                            nc.vector.tensor_copy(out=vb, in_=pb)
                            for s4 in range(4):
                                s = g * 4 + s4
                                ptv = ppt.tile([128, 128], bf16, tag="pt")
                                nc.tensor.transpose(
                                    ptv, vb[:, s4 * 128:(s4 + 1) * 128], idb
                                )
                                nc.vector.tensor_scalar_mul(
                                    out=v_sb[:, s, m * 128:(m + 1) * 128],
                                    in0=ptv,
                                    scalar1=invc_sb[:, s:s + 1],
                                )

            # ---------------- phase 3: hash scores (draft)
            for h in range(HPC):
                rt1s = wp.tile([128, 128], f32, tag="rts")
                nc.sync.dma_start(rt1s, rot1[h])
                rt1 = wp.tile([128, 128], f32r, tag="rt")
                nc.gpsimd.tensor_copy(out=rt1, in_=rt1s)
                rt2s = wp.tile([128, 128], f32, tag="rts")
                nc.sync.dma_start(rt2s, rot2[h])
                rt2 = wp.tile([128, 128], f32r, tag="rt")
                nc.gpsimd.tensor_copy(out=rt2, in_=rt2s)
                hashT = {}
                for side, srcT in (("q", qrT), ("k", krT)):
                    hh = sc2.tile([128, S], f32r, tag=f"hash{side}")
                    for g in range(2):
                        gs = slice(g * 512, (g + 1) * 512)
                        hp1 = pp2.tile([128, 512], f32, tag="pb")
                        nc.tensor.matmul(hp1, rt1, srcT[:, h, gs],
                                         start=True, stop=True)
                        s1 = sc2.tile([128, 512], f32r, tag="s1")
                        nc.scalar.activation(out=s1, in_=hp1,
                                             func=mybir.ActivationFunctionType.Silu)
                        hp2 = pp2.tile([128, 512], f32, tag="pb")
                        nc.tensor.matmul(hp2, rt2, s1, start=True, stop=True)
                        ab = sc1.tile([128, 512], f32, tag="scr1")
                        nc.scalar.activation(out=ab, in_=hp2,
                                             func=mybir.ActivationFunctionType.Abs,
                                             scale=GAMMA)
                        nc.scalar.add(out=ab, in_=ab, add=1.0)
                        rcp = sc1.tile([128, 512], f32, tag="scr1")
                        nc.vector.reciprocal(out=rcp, in_=ab)
                        nc.scalar.mul(out=rcp, in_=rcp, mul=GAMMA)
                        nc.vector.tensor_mul(out=hh[:, gs], in0=hp2, in1=rcp)
                    hashT[side] = hh
                for qc in range(SCH):
                    for g in range(2):
                        dp = pp2.tile([128, 512], f32, tag="pb")
                        nc.tensor.matmul(
                            dp, hashT["q"][:, qc * 128:(qc + 1) * 128],
                            hashT["k"][:, g * 512:(g + 1) * 512],
                            start=True, stop=True,
                        )
                        dcp = bigf.tile([128, 512], f32, tag="bigf")
                        nc.vector.tensor_copy(out=dcp, in_=dp)
                        nc.sync.dma_start(
                            draft[h, qc * 128:(qc + 1) * 128,
                                  g * 512:(g + 1) * 512], dcp
                        )

            # ---------------- phase 4: true causal attention
            for h in range(HPC):
                for qc in range(SCH):
                    ks = (qc + 1) * 128
                    lsb = bigf.tile([128, S], f32, tag="bigf")
                    for n0 in range(0, ks, 512):
                        nn = min(512, ks - n0)
                        lp = pp2.tile([128, 512], f32, tag="pb")
                        nc.tensor.matmul(
                            lp[:, 0:nn],
                            qrT[:, h, qc * 128:(qc + 1) * 128],
                            krT[:, h, n0:n0 + nn], start=True, stop=True,
                        )
                        nc.vector.tensor_copy(out=lsb[:, n0:n0 + nn], in_=lp[:, 0:nn])
                    nc.vector.tensor_add(
                        out=lsb[:, qc * 128:ks], in0=lsb[:, qc * 128:ks], in1=tri_sb
                    )
                    negm = sc2.tile([128, 1], f32, tag="negm")
                    nc.vector.reduce_max(out=negm, in_=lsb[:, 0:ks],
                                         axis=mybir.AxisListType.X, negate=True)
                    negms = sc2.tile([128, 1], f32, tag="negms")
                    nc.scalar.mul(out=negms, in_=negm, mul=ISQ)
                    pu = bigf.tile([128, S], f32, tag="bigf")
                    rsum = sc2.tile([128, 1], f32, tag="rsum")
                    nc.scalar.activation(out=pu[:, 0:ks], in_=lsb[:, 0:ks],
                                         func=mybir.ActivationFunctionType.Exp,
                                         bias=negms, scale=ISQ, accum_out=rsum)
                    rinv = sc2.tile([128, 1], f32, tag="rinv")
                    nc.vector.reciprocal(out=rinv, in_=rsum)
                    pf = bigf.tile([128, S], f32, tag="bigf")
                    nc.vector.tensor_scalar_mul(out=pf[:, 0:ks], in0=pu[:, 0:ks],
                                                scalar1=rinv)
                    nc.sync.dma_start(trueo[h, qc * 128:(qc + 1) * 128, 0:ks],
                                      pf[:, 0:ks])
                    if ks < S:
                        nc.sync.dma_start(trueo[h, qc * 128:(qc + 1) * 128, ks:S],
                                          zt[:, 0:S - ks])
                    pbf = sc2.tile([128, S], bf16, tag="pbf")
                    nc.vector.tensor_scalar_mul(out=pbf[:, 0:ks], in0=pu[:, 0:ks],
                                                scalar1=rinv)
                    ap_ps = ppa.tile([128, 128], f32, tag="ap_ps")
                    for kc in range(qc + 1):
                        ptp = ppt.tile([128, 128], bf16, tag="pt")
                        nc.tensor.transpose(ptp, pbf[:, kc * 128:(kc + 1) * 128], idb)
                        pts = sc2.tile([128, 128], bf16, tag="pts")
                        nc.vector.tensor_copy(out=pts, in_=ptp)
                        nc.tensor.matmul(
                            ap_ps, v_sb[:, kc, h * 128:(h + 1) * 128], pts,
                            start=(kc == 0), stop=(kc == qc),
                        )
                    nc.vector.tensor_copy(
                        out=attnT[:, h, qc * 128:(qc + 1) * 128], in_=ap_ps
                    )

            # ---------------- phase 5: output projection partial
            for n in range(8):
                wots = []
                for hh in range(HPC):
                    wot = wop.tile([128, 512], bf16, tag="wot")
                    nc.sync.dma_start(
                        wot, wo[hh * 128:(hh + 1) * 128, n * 512:(n + 1) * 512]
                    )
                    wots.append(wot)
                for sc in range(SCH):
                    po = pp2.tile([128, 512], f32, tag="pb")
                    for hh in range(HPC):
                        nc.tensor.matmul(
                            po, attnT[:, hh, sc * 128:(sc + 1) * 128], wots[hh],
                            start=(hh == 0), stop=(hh == HPC - 1),
                        )
                    ob = sc2.tile([128, 512], f32, tag="ob")
                    nc.scalar.copy(out=ob, in_=po)
                    nc.sync.dma_start(
                        opart[sc * 128:(sc + 1) * 128, n * 512:(n + 1) * 512], ob
                    )

    nc.compile()
    return nc


# ---------------------------------------------------------------- L2 program
def _build_l2(reps: int = 1):
    nc = bacc.Bacc("TRN2", target_bir_lowering=False, debug=False, num_devices=NC)
    hnT = nc.dram_tensor("hnT", [H, S], bf16, kind="ExternalInput").ap()
    wg = nc.dram_tensor("wg", [H, FFP], bf16, kind="ExternalInput").ap()
    wu = nc.dram_tensor("wu", [H, FFP], bf16, kind="ExternalInput").ap()
    wd = nc.dram_tensor("wd", [FFP, H], bf16, kind="ExternalInput").ap()
    mlp = nc.dram_tensor("mlp", [S, H], f32, kind="ExternalOutput").ap()
    FCH = FFP // 128  # 11

    with tile.TileContext(nc) as tc:
        import contextlib
        with contextlib.ExitStack() as ctx:
            pers = ctx.enter_context(tc.tile_pool(name="pers", bufs=1))
            wp = ctx.enter_context(tc.tile_pool(name="wp", bufs=4))
            wdp = ctx.enter_context(tc.tile_pool(name="wdp", bufs=12))
            sc2 = ctx.enter_context(tc.tile_pool(name="sc2", bufs=2))
            bigf = ctx.enter_context(tc.tile_pool(name="bigf", bufs=5))
            pp2 = ctx.enter_context(tc.tile_pool(name="pp2", bufs=2, space="PSUM"))
            ppo = ctx.enter_context(tc.tile_pool(name="ppo", bufs=2, space="PSUM"))

            hnT_sb = pers.tile([128, KCH, S], bf16)   # 64K
            gated = pers.tile([128, FCH, S], bf16)    # 22K
            if reps > 1:
                ctx.enter_context(tc.For_i(0, reps, 1))
            for k in range(KCH):
                nc.sync.dma_start(hnT_sb[:, k, :], hnT[k * 128:(k + 1) * 128, :])

            for f in range(FCH):
                pg = pp2.tile([128, S], f32, tag="pg")
                for k in range(KCH):
                    wgt = wp.tile([128, 128], bf16, tag="wgt")
                    nc.sync.dma_start(
                        wgt, wg[k * 128:(k + 1) * 128, f * 128:(f + 1) * 128]
                    )
                    for n0 in (0, 512):
                        nc.tensor.matmul(pg[:, n0:n0 + 512], wgt,
                                         hnT_sb[:, k, n0:n0 + 512],
                                         start=(k == 0), stop=(k == KCH - 1))
                sg = sc2.tile([128, S], bf16, tag="sg")
                nc.scalar.activation(out=sg, in_=pg,
                                     func=mybir.ActivationFunctionType.Silu)
                pup = pp2.tile([128, S], f32, tag="pg")
                for k in range(KCH):
                    wut = wp.tile([128, 128], bf16, tag="wgt")
                    nc.sync.dma_start(
                        wut, wu[k * 128:(k + 1) * 128, f * 128:(f + 1) * 128]
                    )
                    for n0 in (0, 512):
                        nc.tensor.matmul(pup[:, n0:n0 + 512], wut,
                                         hnT_sb[:, k, n0:n0 + 512],
                                         start=(k == 0), stop=(k == KCH - 1))
                nc.vector.tensor_mul(out=gated[:, f, :], in0=pup, in1=sg)

            for n in range(8):
                wdts = []
                for f in range(FCH):
                    wdt = wdp.tile([128, 512], bf16, tag="wdt")
                    nc.sync.dma_start(
                        wdt, wd[f * 128:(f + 1) * 128, n * 512:(n + 1) * 512]
                    )
                    wdts.append(wdt)
                for sc in range(SCH):
                    pd = ppo.tile([128, 512], f32, tag="pd")
                    for f in range(FCH):
                        nc.tensor.matmul(
                            pd, gated[:, f, sc * 128:(sc + 1) * 128], wdts[f],
                            start=(f == 0), stop=(f == FCH - 1),
                        )
                    ob = sc2.tile([128, 512], f32, tag="ob")
                    nc.scalar.copy(out=ob, in_=pd)
                    nc.sync.dma_start(
                        mlp[sc * 128:(sc + 1) * 128, n * 512:(n + 1) * 512], ob
                    )

    nc.compile()
    return nc


# ------------------------------------------------------- cached SPMD runner
class _SpmdRunner:
    """Compile a Bass program into a cached jax.jit callable over 8 cores."""

    def __init__(self, nc):
        import jax
        from jax.experimental.shard_map import shard_map
        from jax.sharding import Mesh, PartitionSpec
        from concourse import bass2jax

        bass2jax.install_neuronx_cc_hook()
        part_name = nc.partition_id_tensor.name if nc.partition_id_tensor else None
        self.in_names, self.out_names, out_avals, self.zero_outs = [], [], [], []
        for alloc in nc.m.functions[0].allocations:
            if not isinstance(alloc, mybir.MemoryLocationSet):
                continue
            name = alloc.memorylocations[0].name
            if alloc.kind == "ExternalInput":
                if name != part_name:
                    self.in_names.append(name)
            elif alloc.kind == "ExternalOutput":
                shape = tuple(alloc.tensor_shape)
                dtype = mybir.dt.np(alloc.dtype)
                out_avals.append(jax.core.ShapedArray(shape, dtype))
                self.out_names.append(name)
                self.zero_outs.append(np.zeros(shape, dtype))
        self.out_avals = out_avals
        n_params = len(self.in_names)
        all_names = self.in_names + self.out_names
        if part_name is not None:
            all_names = all_names + [part_name]

        def _body(*args):
            operands = list(args)
            if part_name is not None:
                operands.append(bass2jax.partition_id_tensor())
            outs = bass2jax._bass_exec_p.bind(
                *operands,
                out_avals=tuple(out_avals),
                in_names=tuple(all_names),
                out_names=tuple(self.out_names),
                lowering_input_output_aliases=(),
                sim_require_finite=True,
                sim_require_nnan=True,
                nc=nc,
            )
            return tuple(outs)

        devices = jax.devices()[:NC]
        mesh = Mesh(np.asarray(devices), ("core",))
        n_all = n_params + len(self.out_names)
        self.fn = jax.jit(
            shard_map(
                _body, mesh=mesh,
                in_specs=(PartitionSpec("core"),) * n_all,
                out_specs=(PartitionSpec("core"),) * len(self.out_names),
                check_rep=False,
            ),
            donate_argnums=tuple(range(n_params, n_all)),
            keep_unused=True,
        )

    def __call__(self, in_maps):
        concat_in = [
            np.concatenate([np.asarray(m[name]) for m in in_maps], axis=0)
            for name in self.in_names
        ]
        concat_zero = [
            np.zeros((NC * z.shape[0], *z.shape[1:]), z.dtype) for z in self.zero_outs
        ]
        outs = self.fn(*concat_in, *concat_zero)
        return [
            {
                name: np.asarray(outs[i]).reshape(NC, *self.out_avals[i].shape)[c]
                for i, name in enumerate(self.out_names)
            }
            for c in range(NC)
        ]


_CACHE = {}


def _programs():
    if "r1" not in _CACHE:
        _CACHE["r1"] = _SpmdRunner(_build_l1())
        _CACHE["r2"] = _SpmdRunner(_build_l2())
    return _CACHE["r1"], _CACHE["r2"]


def timing_runners(reps: int):
    key = f"t{reps}"
    if key not in _CACHE:
        _CACHE[key] = (_SpmdRunner(_build_l1(reps)), _SpmdRunner(_build_l2(reps)))
    return _CACHE[key]


# ----------------------------------------------------------- host-side prep
def _rope_cos_sin_T():
    inv_freq = 1.0 / (10000.0 ** (np.arange(0, HD, 2, dtype=np.float64) / HD))
    freqs = np.outer(np.arange(S, dtype=np.float64), inv_freq)
    emb = np.concatenate([freqs, freqs], axis=-1)
    return (np.cos(emb).astype(np.float32).T.copy(),
            np.sin(emb).astype(np.float32).T.copy())


def _l1_inputs(hidden, wq, wk, wv, wo, rot1, rot2, ln1_w):
    hid = np.asarray(hidden, np.float32).reshape(S, H)
    invr = (1.0 / np.sqrt((hid.astype(np.float64) ** 2).mean(-1) + EPS)).astype(
        np.float32
    )
    invc = invr.reshape(SCH, 128).T.copy()          # [i, s]
    cosT, sinT = _rope_cos_sin_T()
    csB = (cosT * invr[None, :]).astype(np.float32)
    snB = (sinT * invr[None, :]).astype(np.float32)
    snB[0:64, :] *= -1.0
    tri = np.triu(np.full((128, 128), NEG, np.float32), k=1)
    wq = (np.asarray(wq, np.float32) * np.asarray(ln1_w, np.float32)[:, None])
    wk = (np.asarray(wk, np.float32) * np.asarray(ln1_w, np.float32)[:, None])
    wv = (np.asarray(wv, np.float32) * np.asarray(ln1_w, np.float32)[:, None])
    wo = np.asarray(wo, np.float32)
    r1 = np.asarray(rot1, np.float32).reshape(NH, HD, HD)
    r2 = np.asarray(rot2, np.float32).reshape(NH, HD, HD)
    maps = []
    for c in range(NC):
        cs = slice(c * DPC, (c + 1) * DPC)
        hs = slice(c * HPC, (c + 1) * HPC)
        maps.append({
            "hid": hid,
            "invr": invr,
            "invc": invc,
            "wq": np.ascontiguousarray(wq[:, cs]),
            "wk": np.ascontiguousarray(wk[:, cs]),
            "wv": np.ascontiguousarray(wv[:, cs]),
            "wo": wo[cs, :].astype(BF),
            "rot1": r1[hs],
            "rot2": r2[hs],
            "csB": csB,
            "snB": snB,
            "tri": tri,
        })
    return maps, hid


def _l2_inputs(h, ln2_w, w_gate, w_up, w_down):
    hn = h * (1.0 / np.sqrt((h.astype(np.float64) ** 2).mean(-1, keepdims=True)
                            + EPS)).astype(np.float32)
    hn = hn * np.asarray(ln2_w, np.float32)[None, :]
    hnT = np.ascontiguousarray(hn.T).astype(BF)
    wg = np.asarray(w_gate, np.float32)
    wu = np.asarray(w_up, np.float32)
    wd = np.asarray(w_down, np.float32)
    maps = []
    for c in range(NC):
        f0 = c * (FF // NC)
        f1 = (c + 1) * (FF // NC)
        wgp = np.zeros((H, FFP), BF)
        wgp[:, : FF // NC] = wg[:, f0:f1].astype(BF)
        wup = np.zeros((H, FFP), BF)
        wup[:, : FF // NC] = wu[:, f0:f1].astype(BF)
        wdp = np.zeros((FFP, H), BF)
        wdp[: FF // NC, :] = wd[f0:f1, :].astype(BF)
        maps.append({"hnT": hnT, "wg": wgp, "wu": wup, "wd": wdp})
    return maps


# ------------------------------------------------------------------- kernel
def kernel(hidden_states, wq, wk, wv, wo, rot_mat1, rot_mat2, ln1_w, ln2_w,
           w_gate, w_up, w_down):
    r1, r2 = _programs()

    maps1, hid = _l1_inputs(hidden_states, wq, wk, wv, wo, rot_mat1, rot_mat2,
                            ln1_w)
    res1 = r1(maps1)

    o_sum = np.sum(np.stack([r["opart"] for r in res1]), axis=0, dtype=np.float32)
    h = hid + o_sum
    maps2 = _l2_inputs(h, ln2_w, w_gate, w_up, w_down)
    res2 = r2(maps2)

    mlp_sum = np.sum(np.stack([r["mlp"] for r in res2]), axis=0, dtype=np.float32)
    out1 = (h + mlp_sum).reshape(B, S, H)
    draft = np.concatenate([r["draft"] for r in res1], axis=0).reshape(B, NH, S, S)
    true = np.concatenate([r["trueo"] for r in res1], axis=0).reshape(B, NH, S, S)
    return out1, draft, true


# revision 11
# speedup vs baseline: 14034.1950x; 26.0783x over previous
"""Trainium2 Bass kernel for nn_Decoder_28922309771884 (sparse_attention decoder layer).

Strategy (8 NeuronCores):
  - Head-parallel attention: 32 heads / 8 cores = 4 heads per core.
    QKV projections column-sharded, wo row-sharded -> per-core partial o.
  - MLP tensor-parallel on the 11008 ffn dim (1376/core, zero-padded to 1408).
  - Two SPMD launches; the cross-core reductions (sum of o partials, sum of
    mlp partials) and the cheap row-stat math (rmsnorm scales, residual adds,
    transposes) run on host between launches.  Device collectives measured
    ~3.6 ms per 16 MiB AllReduce here - far slower than host reduction.
  - All matmuls run with bf16 inputs (fp32 PSUM accumulation).  fp32 data
    feeds every non-matmul stage.
"""

import sys

sys.path.insert(0, "/opt/trn_rl_repo")

import numpy as np
import ml_dtypes

import concourse.bass as bass
import concourse.mybir as mybir
import concourse.tile as tile
from concourse import bacc
from concourse.masks import make_identity

f32 = mybir.dt.float32
f32r = mybir.dt.float32r
bf16 = mybir.dt.bfloat16
BF = ml_dtypes.bfloat16

B, S, H, NH, HD, FF = 1, 1024, 4096, 32, 128, 11008
GAMMA = 64.0
EPS = 1e-5
NC = 8                 # cores
HPC = NH // NC         # heads per core = 4
DPC = HPC * HD         # head dims per core = 512
FFP = 1408             # padded ffn dims per core (1376 -> 11*128)
ISQ = float(1.0 / np.sqrt(HD))
SCH = S // 128         # 8 sequence chunks
KCH = H // 128         # 32 hidden chunks
NEG = -3.0e38


# ---------------------------------------------------------------- L1 program
def _build_l1(reps: int = 1):
    nc = bacc.Bacc("TRN2", target_bir_lowering=False, debug=False, num_devices=NC)
    hid = nc.dram_tensor("hid", [S, H], f32, kind="ExternalInput").ap()
    invr = nc.dram_tensor("invr", [S], f32, kind="ExternalInput").ap()
    invc = nc.dram_tensor("invc", [128, SCH], f32, kind="ExternalInput").ap()
    wq = nc.dram_tensor("wq", [H, DPC], f32, kind="ExternalInput").ap()
    wk = nc.dram_tensor("wk", [H, DPC], f32, kind="ExternalInput").ap()
    wv = nc.dram_tensor("wv", [H, DPC], f32, kind="ExternalInput").ap()
    wo = nc.dram_tensor("wo", [DPC, H], bf16, kind="ExternalInput").ap()
    rot1 = nc.dram_tensor("rot1", [HPC, HD, HD], f32, kind="ExternalInput").ap()
    rot2 = nc.dram_tensor("rot2", [HPC, HD, HD], f32, kind="ExternalInput").ap()
    csB = nc.dram_tensor("csB", [HD, S], f32, kind="ExternalInput").ap()
    snB = nc.dram_tensor("snB", [HD, S], f32, kind="ExternalInput").ap()
    tri = nc.dram_tensor("tri", [128, 128], f32, kind="ExternalInput").ap()

    draft = nc.dram_tensor("draft", [HPC, S, S], f32, kind="ExternalOutput").ap()
    trueo = nc.dram_tensor("trueo", [HPC, S, S], f32, kind="ExternalOutput").ap()
    opart = nc.dram_tensor("opart", [S, H], f32, kind="ExternalOutput").ap()

    with tile.TileContext(nc) as tc:
        import contextlib
        with contextlib.ExitStack() as ctx:
            cp = ctx.enter_context(tc.tile_pool(name="cp", bufs=1))
            pers = ctx.enter_context(tc.tile_pool(name="pers", bufs=1))
            hseg = ctx.enter_context(tc.tile_pool(name="hseg", bufs=1))
            hcp = ctx.enter_context(tc.tile_pool(name="hcp", bufs=4))
            wp = ctx.enter_context(tc.tile_pool(name="wp", bufs=4))
            wop = ctx.enter_context(tc.tile_pool(name="wop", bufs=5))
            sc1 = ctx.enter_context(tc.tile_pool(name="sc1", bufs=4))
            sc2 = ctx.enter_context(tc.tile_pool(name="sc2", bufs=2))
            bigf = ctx.enter_context(tc.tile_pool(name="bigf", bufs=5))
            pp2 = ctx.enter_context(tc.tile_pool(name="pp2", bufs=3, space="PSUM"))
            ppt = ctx.enter_context(tc.tile_pool(name="ppt", bufs=3, space="PSUM"))
            ppa = ctx.enter_context(tc.tile_pool(name="ppa", bufs=1, space="PSUM"))

            # ---------------- constants
            idb = cp.tile([128, 128], bf16)
            make_identity(nc, idb)
            idf = cp.tile([128, 128], f32)
            make_identity(nc, idf)
            idr = cp.tile([128, 128], f32r)
            nc.gpsimd.tensor_copy(out=idr, in_=idf)
            csB_sb = cp.tile([128, S], f32)
            nc.sync.dma_start(csB_sb, csB)
            snB_sb = cp.tile([128, S], f32)
            nc.sync.dma_start(snB_sb, snB)
            tri_sb = cp.tile([128, 128], f32)
            nc.sync.dma_start(tri_sb, tri)
            invc_sb = cp.tile([128, SCH], f32)
            nc.sync.dma_start(invc_sb, invc)
            zt = cp.tile([128, S - 128], f32)
            nc.vector.memset(zt, 0.0)

            # persistent activation tiles (per-partition bytes in comments)
            qrT = pers.tile([128, HPC, S], f32r)         # 16K
            krT = pers.tile([128, HPC, S], f32r)         # 16K
            v_sb = pers.tile([128, SCH, DPC], bf16)      # 8K
            attnT = pers.tile([128, HPC, S], bf16)       # 8K

            if reps > 1:
                ctx.enter_context(tc.For_i(0, reps, 1))

            # ---------------- phase 1+2: QKV in f32r, S processed in halves
            for g in range(2):
                hidT = hseg.tile([128, KCH, 512], f32r, tag="hidT")
                for s4 in range(4):
                    s = g * 4 + s4
                    for k in range(KCH):
                        hc = hcp.tile([128, 128], f32, tag="hc")
                        nc.sync.dma_start(
                            hc, hid[s * 128:(s + 1) * 128, k * 128:(k + 1) * 128]
                        )
                        hr = hcp.tile([128, 128], f32r, tag="hr")
                        nc.gpsimd.tensor_copy(out=hr, in_=hc)
                        ptr = ppt.tile([128, 128], f32r, tag="pt")
                        nc.tensor.transpose(ptr, hr, idr)
                        nc.vector.tensor_copy(
                            out=hidT[:, k, s4 * 128:(s4 + 1) * 128], in_=ptr
                        )
                gs = slice(g * 512, (g + 1) * 512)
                for widx, wap in ((0, wq), (1, wk), (2, wv)):
                    for m in range(HPC):
                        pb = pp2.tile([128, 512], f32, tag="pb")
                        for k in range(KCH):
                            ws = wp.tile([128, 128], f32, tag="ws")
                            nc.sync.dma_start(
                                ws, wap[k * 128:(k + 1) * 128, m * 128:(m + 1) * 128]
                            )
                            wr = wp.tile([128, 128], f32r, tag="wr")
                            nc.gpsimd.tensor_copy(out=wr, in_=ws)
                            nc.tensor.matmul(pb, wr, hidT[:, k, :],
                                             start=(k == 0), stop=(k == KCH - 1))
                        if widx < 2:
                            # rope: qr = q*csB + swap_halves(q)*snB
                            qraw = sc1.tile([128, 512], f32, tag="scr1")
                            nc.vector.tensor_copy(out=qraw, in_=pb)
                            tcos = sc1.tile([128, 512], f32, tag="scr1")
                            nc.vector.tensor_mul(out=tcos, in0=pb, in1=csB_sb[:, gs])
                            qsw = sc1.tile([128, 512], f32, tag="scr1")
                            nc.sync.dma_start(qsw[0:64, :], qraw[64:128, :])
                            nc.sync.dma_start(qsw[64:128, :], qraw[0:64, :])
                            tsin = sc1.tile([128, 512], f32, tag="scr1")
                            nc.vector.tensor_mul(out=tsin, in0=qsw, in1=snB_sb[:, gs])
                            dst = qrT if widx == 0 else krT
                            nc.vector.tensor_add(out=dst[:, m, gs], in0=tcos, in1=tsin)
                        else:
                            # v: cast bf16, transpose to [s, hd], normalize rows
                            vb = sc2.tile([128, 512], bf16, tag="vb")
                            nc.vector.tensor_copy(out=vb, in_=pb)
                            for s4 in range(4):
                                s = g * 4 + s4
                                ptv = ppt.tile([128, 128], bf16, tag="pt")
                                nc.tensor.transpose(
                                    ptv, vb[:, s4 * 128:(s4 + 1) * 128], idb
                                )
                                nc.vector.tensor_scalar_mul(
                                    out=v_sb[:, s, m * 128:(m + 1) * 128],
                                    in0=ptv,
                                    scalar1=invc_sb[:, s:s + 1],
                                )

            # ---------------- phase 3: hash scores (draft)
            for h in range(HPC):
                rt1s = wp.tile([128, 128], f32, tag="rts")
                nc.sync.dma_start(rt1s, rot1[h])
                rt1 = wp.tile([128, 128], f32r, tag="rt")
                nc.gpsimd.tensor_copy(out=rt1, in_=rt1s)
                rt2s = wp.tile([128, 128], f32, tag="rts")
                nc.sync.dma_start(rt2s, rot2[h])
                rt2 = wp.tile([128, 128], f32r, tag="rt")
                nc.gpsimd.tensor_copy(out=rt2, in_=rt2s)
                hashT = {}
                for side, srcT in (("q", qrT), ("k", krT)):
                    hh = sc2.tile([128, S], f32r, tag=f"hash{side}")
                    for g in range(2):
                        gs = slice(g * 512, (g + 1) * 512)
                        hp1 = pp2.tile([128, 512], f32, tag="pb")
                        nc.tensor.matmul(hp1, rt1, srcT[:, h, gs],
                                         start=True, stop=True)
                        s1 = sc2.tile([128, 512], f32r, tag="s1")
                        nc.scalar.activation(out=s1, in_=hp1,
                                             func=mybir.ActivationFunctionType.Silu)
                        hp2 = pp2.tile([128, 512], f32, tag="pb")
                        nc.tensor.matmul(hp2, rt2, s1, start=True, stop=True)
                        ab = sc1.tile([128, 512], f32, tag="scr1")
                        nc.scalar.activation(out=ab, in_=hp2,
                                             func=mybir.ActivationFunctionType.Abs,
                                             scale=GAMMA)
                        nc.scalar.add(out=ab, in_=ab, add=1.0)
                        rcp = sc1.tile([128, 512], f32, tag="scr1")
                        nc.vector.reciprocal(out=rcp, in_=ab)
                        nc.scalar.mul(out=rcp, in_=rcp, mul=GAMMA)
                        nc.vector.tensor_mul(out=hh[:, gs], in0=hp2, in1=rcp)
                    hashT[side] = hh
                for qc in range(SCH):
                    for g in range(2):
                        dp = pp2.tile([128, 512], f32, tag="pb")
                        nc.tensor.matmul(
                            dp, hashT["q"][:, qc * 128:(qc + 1) * 128],
                            hashT["k"][:, g * 512:(g + 1) * 512],
                            start=True, stop=True,
                        )
                        dcp = bigf.tile([128, 512], f32, tag="bigf")
                        nc.vector.tensor_copy(out=dcp, in_=dp)
                        nc.sync.dma_start(
                            draft[h, qc * 128:(qc + 1) * 128,
                                  g * 512:(g + 1) * 512], dcp
                        )

            # ---------------- phase 4: true causal attention
            for h in range(HPC):
                for qc in range(SCH):
                    ks = (qc + 1) * 128
                    lsb = bigf.tile([128, S], f32, tag="bigf")
                    for n0 in range(0, ks, 512):
                        nn = min(512, ks - n0)
                        lp = pp2.tile([128, 512], f32, tag="pb")
                        nc.tensor.matmul(
                            lp[:, 0:nn],
                            qrT[:, h, qc * 128:(qc + 1) * 128],
                            krT[:, h, n0:n0 + nn], start=True, stop=True,
                        )
                        nc.vector.tensor_copy(out=lsb[:, n0:n0 + nn], in_=lp[:, 0:nn])
                    nc.vector.tensor_add(
                        out=lsb[:, qc * 128:ks], in0=lsb[:, qc * 128:ks], in1=tri_sb
                    )
                    negm = sc2.tile([128, 1], f32, tag="negm")
                    nc.vector.reduce_max(out=negm, in_=lsb[:, 0:ks],
                                         axis=mybir.AxisListType.X, negate=True)
                    negms = sc2.tile([128, 1], f32, tag="negms")
                    nc.scalar.mul(out=negms, in_=negm, mul=ISQ)
                    pu = bigf.tile([128, S], f32, tag="bigf")
                    rsum = sc2.tile([128, 1], f32, tag="rsum")
                    nc.scalar.activation(out=pu[:, 0:ks], in_=lsb[:, 0:ks],
                                         func=mybir.ActivationFunctionType.Exp,
                                         bias=negms, scale=ISQ, accum_out=rsum)
                    rinv = sc2.tile([128, 1], f32, tag="rinv")
                    nc.vector.reciprocal(out=rinv, in_=rsum)
                    pf = bigf.tile([128, S], f32, tag="bigf")
                    nc.vector.tensor_scalar_mul(out=pf[:, 0:ks], in0=pu[:, 0:ks],
                                                scalar1=rinv)
                    nc.sync.dma_start(trueo[h, qc * 128:(qc + 1) * 128, 0:ks],
                                      pf[:, 0:ks])
                    if ks < S:
                        nc.sync.dma_start(trueo[h, qc * 128:(qc + 1) * 128, ks:S],
                                          zt[:, 0:S - ks])
                    pbf = sc2.tile([128, S], bf16, tag="pbf")
                    nc.vector.tensor_scalar_mul(out=pbf[:, 0:ks], in0=pu[:, 0:ks],
                                                scalar1=rinv)
                    ap_ps = ppa.tile([128, 128], f32, tag="ap_ps")
                    for kc in range(qc + 1):
                        ptp = ppt.tile([128, 128], bf16, tag="pt")
                        nc.tensor.transpose(ptp, pbf[:, kc * 128:(kc + 1) * 128], idb)
                        pts = sc2.tile([128, 128], bf16, tag="pts")
                        nc.vector.tensor_copy(out=pts, in_=ptp)
                        nc.tensor.matmul(
                            ap_ps, v_sb[:, kc, h * 128:(h + 1) * 128], pts,
                            start=(kc == 0), stop=(kc == qc),
                        )
                    nc.vector.tensor_copy(
                        out=attnT[:, h, qc * 128:(qc + 1) * 128], in_=ap_ps
                    )

            # ---------------- phase 5: output projection partial
            for n in range(8):
                wots = []
                for hh in range(HPC):
                    wot = wop.tile([128, 512], bf16, tag="wot")
                    nc.sync.dma_start(
                        wot, wo[hh * 128:(hh + 1) * 128, n * 512:(n + 1) * 512]
                    )
                    wots.append(wot)
                for sc in range(SCH):
                    po = pp2.tile([128, 512], f32, tag="pb")
                    for hh in range(HPC):
                        nc.tensor.matmul(
                            po, attnT[:, hh, sc * 128:(sc + 1) * 128], wots[hh],
                            start=(hh == 0), stop=(hh == HPC - 1),
                        )
                    ob = sc2.tile([128, 512], f32, tag="ob")
                    nc.scalar.copy(out=ob, in_=po)
                    nc.sync.dma_start(
                        opart[sc * 128:(sc + 1) * 128, n * 512:(n + 1) * 512], ob
                    )

    nc.compile()
    return nc


# ---------------------------------------------------------------- L2 program
def _build_l2(reps: int = 1):
    nc = bacc.Bacc("TRN2", target_bir_lowering=False, debug=False, num_devices=NC)
    hnT = nc.dram_tensor("hnT", [H, S], bf16, kind="ExternalInput").ap()
    wg = nc.dram_tensor("wg", [H, FFP], bf16, kind="ExternalInput").ap()
    wu = nc.dram_tensor("wu", [H, FFP], bf16, kind="ExternalInput").ap()
    wd = nc.dram_tensor("wd", [FFP, H], bf16, kind="ExternalInput").ap()
    mlp = nc.dram_tensor("mlp", [S, H], f32, kind="ExternalOutput").ap()
    FCH = FFP // 128  # 11

    with tile.TileContext(nc) as tc:
        import contextlib
        with contextlib.ExitStack() as ctx:
            pers = ctx.enter_context(tc.tile_pool(name="pers", bufs=1))
            wp = ctx.enter_context(tc.tile_pool(name="wp", bufs=4))
            wdp = ctx.enter_context(tc.tile_pool(name="wdp", bufs=12))
            sc2 = ctx.enter_context(tc.tile_pool(name="sc2", bufs=2))
            bigf = ctx.enter_context(tc.tile_pool(name="bigf", bufs=5))
            pp2 = ctx.enter_context(tc.tile_pool(name="pp2", bufs=2, space="PSUM"))
            ppo = ctx.enter_context(tc.tile_pool(name="ppo", bufs=2, space="PSUM"))

            hnT_sb = pers.tile([128, KCH, S], bf16)   # 64K
            gated = pers.tile([128, FCH, S], bf16)    # 22K
            if reps > 1:
                ctx.enter_context(tc.For_i(0, reps, 1))
            for k in range(KCH):
                nc.sync.dma_start(hnT_sb[:, k, :], hnT[k * 128:(k + 1) * 128, :])

            for f in range(FCH):
                pg = pp2.tile([128, S], f32, tag="pg")
                for k in range(KCH):
                    wgt = wp.tile([128, 128], bf16, tag="wgt")
                    nc.sync.dma_start(
                        wgt, wg[k * 128:(k + 1) * 128, f * 128:(f + 1) * 128]
                    )
                    for n0 in (0, 512):
                        nc.tensor.matmul(pg[:, n0:n0 + 512], wgt,
                                         hnT_sb[:, k, n0:n0 + 512],
                                         start=(k == 0), stop=(k == KCH - 1))
                sg = sc2.tile([128, S], bf16, tag="sg")
                nc.scalar.activation(out=sg, in_=pg,
                                     func=mybir.ActivationFunctionType.Silu)
                pup = pp2.tile([128, S], f32, tag="pg")
                for k in range(KCH):
                    wut = wp.tile([128, 128], bf16, tag="wgt")
                    nc.sync.dma_start(
                        wut, wu[k * 128:(k + 1) * 128, f * 128:(f + 1) * 128]
                    )
                    for n0 in (0, 512):
                        nc.tensor.matmul(pup[:, n0:n0 + 512], wut,
                                         hnT_sb[:, k, n0:n0 + 512],
                                         start=(k == 0), stop=(k == KCH - 1))
                nc.vector.tensor_mul(out=gated[:, f, :], in0=pup, in1=sg)

            for n in range(8):
                wdts = []
                for f in range(FCH):
                    wdt = wdp.tile([128, 512], bf16, tag="wdt")
                    nc.sync.dma_start(
                        wdt, wd[f * 128:(f + 1) * 128, n * 512:(n + 1) * 512]
                    )
                    wdts.append(wdt)
                for sc in range(SCH):
                    pd = ppo.tile([128, 512], f32, tag="pd")
                    for f in range(FCH):
                        nc.tensor.matmul(
                            pd, gated[:, f, sc * 128:(sc + 1) * 128], wdts[f],
                            start=(f == 0), stop=(f == FCH - 1),
                        )
                    ob = sc2.tile([128, 512], f32, tag="ob")
                    nc.scalar.copy(out=ob, in_=pd)
                    nc.sync.dma_start(
                        mlp[sc * 128:(sc + 1) * 128, n * 512:(n + 1) * 512], ob
                    )

    nc.compile()
    return nc


# ------------------------------------------------------- cached SPMD runner
class _SpmdRunner:
    """Compile a Bass program into a cached jax.jit callable over 8 cores."""

    def __init__(self, nc):
        import jax
        from jax.experimental.shard_map import shard_map
        from jax.sharding import Mesh, PartitionSpec
        from concourse import bass2jax

        bass2jax.install_neuronx_cc_hook()
        part_name = nc.partition_id_tensor.name if nc.partition_id_tensor else None
        self.in_names, self.out_names, out_avals, self.zero_outs = [], [], [], []
        for alloc in nc.m.functions[0].allocations:
            if not isinstance(alloc, mybir.MemoryLocationSet):
                continue
            name = alloc.memorylocations[0].name
            if alloc.kind == "ExternalInput":
                if name != part_name:
                    self.in_names.append(name)
            elif alloc.kind == "ExternalOutput":
                shape = tuple(alloc.tensor_shape)
                dtype = mybir.dt.np(alloc.dtype)
                out_avals.append(jax.core.ShapedArray(shape, dtype))
                self.out_names.append(name)
                self.zero_outs.append(np.zeros(shape, dtype))
        self.out_avals = out_avals
        n_params = len(self.in_names)
        all_names = self.in_names + self.out_names
        if part_name is not None:
            all_names = all_names + [part_name]

        def _body(*args):
            operands = list(args)
            if part_name is not None:
                operands.append(bass2jax.partition_id_tensor())
            outs = bass2jax._bass_exec_p.bind(
                *operands,
                out_avals=tuple(out_avals),
                in_names=tuple(all_names),
                out_names=tuple(self.out_names),
                lowering_input_output_aliases=(),
                sim_require_finite=True,
                sim_require_nnan=True,
                nc=nc,
            )
            return tuple(outs)

        devices = jax.devices()[:NC]
        mesh = Mesh(np.asarray(devices), ("core",))
        self.mesh = mesh
        n_all = n_params + len(self.out_names)
        self.fn = jax.jit(
            shard_map(
                _body, mesh=mesh,
                in_specs=(PartitionSpec("core"),) * n_all,
                out_specs=(PartitionSpec("core"),) * len(self.out_names),
                check_rep=False,
            ),
            donate_argnums=tuple(range(n_params, n_all)),
            keep_unused=True,
        )

    def device_inputs(self, in_maps):
        import jax
        from jax.sharding import NamedSharding, PartitionSpec
        sh = NamedSharding(self.mesh, PartitionSpec("core"))
        return [
            jax.device_put(
                np.concatenate([np.asarray(m[name]) for m in in_maps], axis=0), sh)
            for name in self.in_names
        ]

    def make_dev_zeros(self):
        import jax, jax.numpy as jnp
        from jax.sharding import NamedSharding, PartitionSpec
        sh = NamedSharding(self.mesh, PartitionSpec("core"))
        if not hasattr(self, "_zfn"):
            shapes = [(NC * z.shape[0], *z.shape[1:]) for z in self.zero_outs]
            dts = [z.dtype for z in self.zero_outs]
            self._zfn = jax.jit(
                lambda: tuple(jnp.zeros(s, d) for s, d in zip(shapes, dts)),
                out_shardings=tuple(sh for _ in shapes))
        return self._zfn()

    def timed_exec(self, dev_in, n=10):
        import time as _t, jax
        ts = []
        for _ in range(n):
            zs = self.make_dev_zeros()
            jax.block_until_ready(zs)
            t0 = _t.time()
            out = self.fn(*dev_in, *zs)
            jax.block_until_ready(out)
            ts.append(_t.time() - t0)
        return min(ts)

    def __call__(self, in_maps):
        concat_in = [
            np.concatenate([np.asarray(m[name]) for m in in_maps], axis=0)
            for name in self.in_names
        ]
        concat_zero = [
            np.zeros((NC * z.shape[0], *z.shape[1:]), z.dtype) for z in self.zero_outs
        ]
        outs = self.fn(*concat_in, *concat_zero)
        return [
            {
                name: np.asarray(outs[i]).reshape(NC, *self.out_avals[i].shape)[c]
                for i, name in enumerate(self.out_names)
            }
            for c in range(NC)
        ]


_CACHE = {}


def _programs():
    if "r1" not in _CACHE:
        _CACHE["r1"] = _SpmdRunner(_build_l1())
        _CACHE["r2"] = _SpmdRunner(_build_l2())
    return _CACHE["r1"], _CACHE["r2"]


def timing_runners(reps: int):
    key = f"t{reps}"
    if key not in _CACHE:
        _CACHE[key] = (_SpmdRunner(_build_l1(reps)), _SpmdRunner(_build_l2(reps)))
    return _CACHE[key]


# ----------------------------------------------------------- host-side prep
def _rope_cos_sin_T():
    inv_freq = 1.0 / (10000.0 ** (np.arange(0, HD, 2, dtype=np.float64) / HD))
    freqs = np.outer(np.arange(S, dtype=np.float64), inv_freq)
    emb = np.concatenate([freqs, freqs], axis=-1)
    return (np.cos(emb).astype(np.float32).T.copy(),
            np.sin(emb).astype(np.float32).T.copy())


def _l1_inputs(hidden, wq, wk, wv, wo, rot1, rot2, ln1_w):
    hid = np.asarray(hidden, np.float32).reshape(S, H)
    invr = (1.0 / np.sqrt((hid.astype(np.float64) ** 2).mean(-1) + EPS)).astype(
        np.float32
    )
    invc = invr.reshape(SCH, 128).T.copy()          # [i, s]
    cosT, sinT = _rope_cos_sin_T()
    csB = (cosT * invr[None, :]).astype(np.float32)
    snB = (sinT * invr[None, :]).astype(np.float32)
    snB[0:64, :] *= -1.0
    tri = np.triu(np.full((128, 128), NEG, np.float32), k=1)
    wq = (np.asarray(wq, np.float32) * np.asarray(ln1_w, np.float32)[:, None])
    wk = (np.asarray(wk, np.float32) * np.asarray(ln1_w, np.float32)[:, None])
    wv = (np.asarray(wv, np.float32) * np.asarray(ln1_w, np.float32)[:, None])
    wo = np.asarray(wo, np.float32)
    r1 = np.asarray(rot1, np.float32).reshape(NH, HD, HD)
    r2 = np.asarray(rot2, np.float32).reshape(NH, HD, HD)
    maps = []
    for c in range(NC):
        cs = slice(c * DPC, (c + 1) * DPC)
        hs = slice(c * HPC, (c + 1) * HPC)
        maps.append({
            "hid": hid,
            "invr": invr,
            "invc": invc,
            "wq": np.ascontiguousarray(wq[:, cs]),
            "wk": np.ascontiguousarray(wk[:, cs]),
            "wv": np.ascontiguousarray(wv[:, cs]),
            "wo": wo[cs, :].astype(BF),
            "rot1": r1[hs],
            "rot2": r2[hs],
            "csB": csB,
            "snB": snB,
            "tri": tri,
        })
    return maps, hid


def _l2_inputs(h, ln2_w, w_gate, w_up, w_down):
    hn = h * (1.0 / np.sqrt((h.astype(np.float64) ** 2).mean(-1, keepdims=True)
                            + EPS)).astype(np.float32)
    hn = hn * np.asarray(ln2_w, np.float32)[None, :]
    hnT = np.ascontiguousarray(hn.T).astype(BF)
    wg = np.asarray(w_gate, np.float32)
    wu = np.asarray(w_up, np.float32)
    wd = np.asarray(w_down, np.float32)
    maps = []
    for c in range(NC):
        f0 = c * (FF // NC)
        f1 = (c + 1) * (FF // NC)
        wgp = np.zeros((H, FFP), BF)
        wgp[:, : FF // NC] = wg[:, f0:f1].astype(BF)
        wup = np.zeros((H, FFP), BF)
        wup[:, : FF // NC] = wu[:, f0:f1].astype(BF)
        wdp = np.zeros((FFP, H), BF)
        wdp[: FF // NC, :] = wd[f0:f1, :].astype(BF)
        maps.append({"hnT": hnT, "wg": wgp, "wu": wup, "wd": wdp})
    return maps


# ------------------------------------------------------------------- kernel
def kernel(hidden_states, wq, wk, wv, wo, rot_mat1, rot_mat2, ln1_w, ln2_w,
           w_gate, w_up, w_down):
    r1, r2 = _programs()

    maps1, hid = _l1_inputs(hidden_states, wq, wk, wv, wo, rot_mat1, rot_mat2,
                            ln1_w)
    res1 = r1(maps1)

    o_sum = np.sum(np.stack([r["opart"] for r in res1]), axis=0, dtype=np.float32)
    h = hid + o_sum
    maps2 = _l2_inputs(h, ln2_w, w_gate, w_up, w_down)
    res2 = r2(maps2)

    mlp_sum = np.sum(np.stack([r["mlp"] for r in res2]), axis=0, dtype=np.float32)
    out1 = (h + mlp_sum).reshape(B, S, H)
    draft = np.concatenate([r["draft"] for r in res1], axis=0).reshape(B, NH, S, S)
    true = np.concatenate([r["trueo"] for r in res1], axis=0).reshape(B, NH, S, S)
    return out1, draft, true
